# revision 66
# baseline (speedup 1.0000x reference)
"""Trainium2 Bass kernel for DiscreteTimeS4.

Reference computation (per batch element b):
    x_proj = relu(x @ Wi^T + bi)          [T, P]
    u      = x_proj @ B                   [T, H]
    h_t    = a * h_{t-1} + u_t            (diagonal linear scan over T)
    y      = hs @ C                       [T, P]
    out    = y @ Wo^T + bo                [T, O]

Sharding: data-parallel over the batch axis -- core b handles x_seq[b].
Weights replicated. No cross-device communication.

Device strategy (per core):
  - host fuses W2 = C @ Wo^T; matmuls run in fp16 (fp32 PSUM), the scan
    in exact fp32 (decay/bias constants fp32).
  - HAM clock gate: the PE runs at 1.2 GHz until it has been busy for a
    full ~3.4us activity window.  A contiguous block of dummy 128-col
    matmuls at the top of the body flips the gate to 2.4 GHz before the
    first real matmul; once warm, sub-window stalls do not re-throttle.
  - per time-chunk c (width w, slow-ramp widths so early data deadlines
    track the bandwidth-bound DMA delivery curve):
      MM1: XP^T[p,t] = Wi @ x^T          (lhsT = Wi^T tile, K=512)
      ACT: relu(psum + bi) -> xp (fp16)
      MM3 of chunk c-2 (transposed): OUT^T[o,t] = W2^T @ HS^T -- issued
           between MM1(c) and MM2(c) so the PE chews on it while Scalar
           runs relu(c); bo bias per-partition: ACT Identity-with-bias
           (m=0,1) and DVE tensor_add vs broadcast bias (m=2,3).
      MM2: U^T[h,t] = B^T @ XP^T         (lhsT = B tile, K=256)
      DVE: tensor_tensor_scan h = a*h + u along t (fp32 state, carry
           chained across chunks, decay broadcast via zero-stride AP)
  - DMA: HBM bandwidth is shared across queues and trigger instructions
    block the issuing engine while its HWDGE ring is full, so: Sync
    (idle engine) carries [Wi k0|ch0] + the x stream in deadline order
    (back-half configs emitted in-loop so ring order paces them) + all
    stores; Scalar carries only Wi k1-3, B, W2; mpack (descriptor-bound
    32B lines) rides the GpSimd ring.  out ships fp16 chunk-major.
"""

import numpy as np

try:
    import concourse.bass as bass
except ImportError:  # pragma: no cover
    import sys

    sys.path.insert(0, "/opt/trn_rl_repo")
    import concourse.bass as bass

from contextlib import ExitStack

import concourse.mybir as mybir
import concourse.tile as tile
from concourse import bacc
from concourse.bass_utils import run_bass_kernel_spmd

BSZ, T, D, P, H, O = 8, 4096, 512, 256, 256, 512
F32 = mybir.dt.float32
F16 = mybir.dt.float16

KD = D // 128  # 4 k-tiles for MM1
KP = P // 128  # 2
KH = H // 128  # 2
MO = O // 128  # 4 output m-tiles for MM3 (transposed layout)

# time-chunk widths: slow ramp so the early chunks' data deadlines track
# the bandwidth-bound DMA delivery curve, tapered tail so the post-scan
# drain is short
WIDTHS = (128, 128, 256, 256, 512, 512, 512, 512, 512, 512, 256)
# PE warm-up: the HAM clock gate runs the PE at 1.2 GHz until it has
# been busy for a full ~3.4us activity window.  Dummy 128-col matmuls
# at the top of the body keep the PE busy from the moment the engines
# come up, so the gate flips to 2.4 GHz right as the first x chunk
# lands instead of ~15us in.  The extra bursts fill the predicted DMA
# stall slots during the ramp (waiting for Wi k1-3 / B / ch1).
N_WARM = 34
W_AFTER_K0 = 0
W_BEFORE_MM2 = 6
W_BEFORE_C1 = 10

_NC_CACHE = {}


def _bcast_free(ap, n):
    """Broadcast a [128, 1] AP along the free dim via zero stride."""
    return bass.AP(tensor=ap.tensor, offset=ap.offset, ap=[list(ap.ap[0]), [0, n]])


def build_nc(widths=WIDTHS):
    key = widths
    if key in _NC_CACHE:
        return _NC_CACHE[key]
    nch = len(widths)
    toffs = [sum(widths[:i]) for i in range(nch)]
    assert sum(widths) == T
    CHMAX = max(widths)
    WI_F = KD * P  # wi block cols total
    # xw layout: [Wi k0 | ch0 | Wi k1-3 | ch1 | ch2 | ...] -- the first
    # load config covers exactly what the chunk-0 k=0 matmuls need in ONE
    # completion signal, so the PE starts as early as possible.
    W0C = KD * widths[0]  # ch0 block cols
    xoff = [P] + [WI_F + KD * t for t in toffs[1:]]  # chunk block offsets

    nc = bacc.Bacc("TRN2", target_bir_lowering=False, debug=False)

    xw_d = nc.dram_tensor("xw", [128, WI_F + KD * T], F16, kind="ExternalInput")
    wb16_d = nc.dram_tensor("wb16", [128, KP * H + KH * O], F16, kind="ExternalInput")
    # fp32 per-partition constants: bi (KP cols), a (KH cols), bo (MO cols)
    mpack_d = nc.dram_tensor("mpack", [128, KP + KH + MO], F32, kind="ExternalInput")
    # chunk-major fp16 output: chunk c occupies cols [MO*toffs[c], MO*toffs[c+1])
    out_d = nc.dram_tensor("out", [128, MO * T], F16, kind="ExternalOutput")

    with tile.TileContext(nc) as tc, ExitStack() as ctx:
        wpool = ctx.enter_context(tc.tile_pool(name="weights", bufs=1))
        xpool = ctx.enter_context(tc.tile_pool(name="x", bufs=1))
        xppool = ctx.enter_context(tc.tile_pool(name="xp", bufs=3))
        hspool = ctx.enter_context(tc.tile_pool(name="hs", bufs=3))
        opool = ctx.enter_context(tc.tile_pool(name="osb", bufs=3))
        psA = ctx.enter_context(tc.tile_pool(name="psA", bufs=2, space="PSUM"))
        psB = ctx.enter_context(tc.tile_pool(name="psB", bufs=2, space="PSUM"))
        # psO tiles hold an (m, m+1) pair: 2 banks each, 2 bufs
        psO = ctx.enter_context(tc.tile_pool(name="psO", bufs=2, space="PSUM"))

        # ---- load configs, two HWDGE streams, head interleaved so the
        # first chunks land as early as possible:
        #   Sync ring:   ch0, Wi(k0), Wi(k1-3), ch1, ch3, then all stores
        #   Scalar ring: mpack, B, ch2, W2, ch4..ch9
        # (stores stay on Sync: the Sync engine is otherwise idle, so the
        # ~600ns trigger instructions never delay relu/bias on Scalar)
        xw_sb = xpool.tile([128, WI_F + KD * T], F16, name="xw_sb")
        # warm-up source tile: memset early on vector (idle at the top)
        warm_sb = wpool.tile([128, 128], F16)
        nc.vector.memset(warm_sb, 0.0)

        def load_chunk(eng, c):
            lo, hi = xoff[c], xoff[c] + KD * widths[c]
            eng.dma_start(out=xw_sb[:, lo:hi], in_=xw_d.ap()[:, lo:hi])

        # DMA trigger instructions BLOCK the issuing engine while the
        # HWDGE ring is full, so the engine whose compute ops gate the PE
        # (Scalar: relu) must carry almost no configs.  Ring plan:
        #   Sync  (idle engine): head + ch1-3, then all stores
        #   Scalar (relu/bias):  mpack, B, W2 only (~2us of triggers)
        #   GpSimd (idle, SWDGE): back-half x chunks (deadlines 20us+)
        # config 1: Wi k0 + ch0 (contiguous), config 2: Wi k1-3
        # HBM bandwidth is SHARED across queues, so the head must not
        # compete with back-half x bytes: everything on the Sync ring in
        # deadline order (ring serialization = pacing), back-half chunk
        # configs emitted inside the loop so they queue behind the head.
        hi0 = P + W0C
        nc.sync.dma_start(out=xw_sb[:, :hi0], in_=xw_d.ap()[:, :hi0])
        load_chunk(nc.sync, 1)
        load_chunk(nc.sync, 2)

        # mpack is tiny but descriptor-gen-bound (128 x 28B lines): on its
        # own GpSimd ring it never poisons an HWDGE queue's head
        mpack_sb = wpool.tile([128, KP + KH + MO], F32)
        nc.gpsimd.dma_start(out=mpack_sb, in_=mpack_d.ap())
        nc.scalar.dma_start(
            out=xw_sb[:, hi0 : hi0 + WI_F - P], in_=xw_d.ap()[:, hi0 : hi0 + WI_F - P]
        )
        wb16_sb = wpool.tile([128, KP * H + KH * O], F16)
        bcols = KP * H
        # B is packed m-major and split per m so MM2(c0) m=0 can start as
        # soon as the first 65KB lands (B completing late on the slow
        # early Scalar queue was a recurring ~1.5us PE stall)
        nc.scalar.dma_start(
            out=wb16_sb[:, : bcols // 2], in_=wb16_d.ap()[:, : bcols // 2]
        )
        nc.scalar.dma_start(
            out=wb16_sb[:, bcols // 2 : bcols], in_=wb16_d.ap()[:, bcols // 2 : bcols]
        )
        # W2 rides the Sync ring between ch2 and ch3: MM3(c0) issues at
        # ~13.6us (right after MM1(c2)), and the weight queue can't land
        # W2 until ~15us -- on Sync it lands ~13.3us, killing a recurring
        # ~1.2us PE stall
        nc.sync.dma_start(out=wb16_sb[:, bcols:], in_=wb16_d.ap()[:, bcols:])
        load_chunk(nc.sync, 3)
        load_chunk(nc.sync, 4)

        # ---- PE warm-up: dummy matmuls into a scratch psO tile.  Same
        # engine + same lhsT back-to-back, so they issue at ~107ns each
        # (cold); interleaved bursts below fill the predicted head-DMA
        # stall slots so the HAM activity window stays saturated and the
        # clock gate flips to 2.4 GHz during the ramp-up, not 15us in.
        warm_ps = psO.tile([128, 2, CHMAX], F32, tag="ps3", name="warm_ps")
        warm_i = [0]

        def warm(n):
            for _ in range(n):
                nc.tensor.matmul(
                    warm_ps[:, warm_i[0] % 2, :128],
                    warm_sb,
                    warm_sb,
                    start=True,
                    stop=True,
                )
                warm_i[0] += 1

        warm(N_WARM)

        bicol = [mpack_sb[:, m : m + 1] for m in range(KP)]
        acol = [mpack_sb[:, KP + m : KP + m + 1] for m in range(KH)]
        bocol = [mpack_sb[:, KP + KH + m : KP + KH + m + 1] for m in range(MO)]

        # broadcast bo[m=2,3] into a [128, 2, CHMAX] fp32 tile so the DVE
        # epilogue handles that m-pair in a single tensor_tensor add
        borep_sb = wpool.tile([128, 2, CHMAX], F32)
        for i, m in enumerate((2, 3)):
            nc.scalar.activation(
                out=borep_sb[:, i, :],
                in_=_bcast_free(bocol[m], CHMAX),
                func=mybir.ActivationFunctionType.Identity,
                bias=bocol[m],
                scale=0.0,
            )

        def wiT_sl(k, m):  # lhsT tile [128, 128] for MM1
            base = k * P if k == 0 else W0C + k * P
            return xw_sb[:, base + m * 128 : base + (m + 1) * 128]

        def x_sl(c, k):  # rhs [128, w] for MM1
            base = xoff[c] + k * widths[c]
            return xw_sb[:, base : base + widths[c]]

        def bmat_sl(k, m):  # lhsT tile for MM2 (B packed m-major)
            base = m * (KP * 128) + k * 128
            return wb16_sb[:, base : base + 128]

        def w2T_sl(k, m):  # lhsT tile for MM3 (transposed output layout)
            base = KP * H + k * O
            return wb16_sb[:, base + m * 128 : base + (m + 1) * 128]

        hs_tiles = [None] * nch

        def mm3_block(c):
            w = widths[c]
            hs_sb = hs_tiles[c]
            # flat [128, MO*w] so the store is contiguous on both sides
            o_sb = opool.tile([128, MO * CHMAX], F16, name=f"o_sb{c}", tag="o_sb")
            for mp in range(MO // 2):  # m-pairs (0,1) and (2,3)
                ps3 = psO.tile([128, 2, CHMAX], F32, tag="ps3", name=f"ps3_{c}_{mp}")
                for mh in range(2):
                    m = 2 * mp + mh
                    for k in range(KH):
                        nc.tensor.matmul(
                            ps3[:, mh, :w],
                            w2T_sl(k, m),
                            hs_sb[:, k, :w],
                            start=(k == 0),
                            stop=(k == KH - 1),
                        )
                if mp == 0:
                    # ACT: per-partition bias, one op per m
                    for mh in range(2):
                        m = 2 * mp + mh
                        nc.scalar.activation(
                            out=o_sb[:, m * w : (m + 1) * w],
                            in_=ps3[:, mh, :w],
                            func=mybir.ActivationFunctionType.Identity,
                            bias=bocol[m],
                            scale=1.0,
                        )
                else:
                    # DVE: whole pair in one add against the broadcast bias
                    nc.vector.tensor_add(
                        o_sb[:, 2 * w : 4 * w], ps3[:, :, :w], borep_sb[:, :, :w]
                    )
            base = MO * toffs[c]
            nc.sync.dma_start(
                out=out_d.ap()[:, base : base + MO * w], in_=o_sb[:, : MO * w]
            )

        for c in range(nch):
            w = widths[c]
            # pace the back-half x loads: chunk c+5's config queues on the
            # Sync ring behind everything this iteration needs
            if 5 <= c + 5 < nch:
                load_chunk(nc.sync, c + 5)
            if c == 1:
                warm(W_BEFORE_C1)

            # ---- MM1 + relu/bias -> xp (fp16)
            # chunk 0 borrows a pair tile from the (still idle) psO pool
            # so chunk 1's MM1 never waits on chunk 0's relu to free psA.
            # c0 runs k-outer: its k=0 matmuls need only the small Wi(k0)
            # config, so the PE starts before the rest of Wi lands (warm
            # bursts cover the predicted DMA stalls in between).
            xp_sb = xppool.tile([128, KP, CHMAX], F16, name=f"xp_sb{c}", tag="xp_sb")
            if c == 0:
                head_pair = psO.tile([128, 2, CHMAX], F32, tag="ps3", name="ps1_head")
                for k in range(KD):
                    for m in range(KP):
                        nc.tensor.matmul(
                            head_pair[:, m, :w],
                            wiT_sl(k, m),
                            x_sl(c, k),
                            start=(k == 0),
                            stop=(k == KD - 1),
                        )
                    if k == 0:
                        warm(W_AFTER_K0)
                for m in range(KP):
                    nc.scalar.activation(
                        out=xp_sb[:, m, :w],
                        in_=head_pair[:, m, :w],
                        func=mybir.ActivationFunctionType.Relu,
                        bias=bicol[m],
                        scale=1.0,
                    )
                warm(W_BEFORE_MM2)
            else:
                for m in range(KP):
                    ps1 = psA.tile([128, CHMAX], F32, tag="ps1", name=f"ps1_{c}_{m}")
                    for k in range(KD):
                        nc.tensor.matmul(
                            ps1[:, :w],
                            wiT_sl(k, m),
                            x_sl(c, k),
                            start=(k == 0),
                            stop=(k == KD - 1),
                        )
                    nc.scalar.activation(
                        out=xp_sb[:, m, :w],
                        in_=ps1[:, :w],
                        func=mybir.ActivationFunctionType.Relu,
                        bias=bicol[m],
                        scale=1.0,
                    )

            # ---- MM3 of chunk c-2 BEFORE MM2(c): while Scalar runs
            # relu(c), the PE chews on the deferred MM3 instead of
            # stalling on xp(c).  (Deferring two chunks also pushes the
            # W2 need-by time out of the bandwidth-critical head window.)
            if c > 1:
                mm3_block(c - 2)

            # ---- MM2 + scan -> hs (fp16, fp32 carry)
            hs_sb = hspool.tile([128, KH, CHMAX], F16, name=f"hs_sb{c}", tag="hs_sb")
            for m in range(KH):
                ps2 = psB.tile([128, CHMAX], F32, tag="ps2", name=f"ps2_{c}_{m}")
                for k in range(KP):
                    nc.tensor.matmul(
                        ps2[:, :w],
                        bmat_sl(k, m),
                        xp_sb[:, k, :w],
                        start=(k == 0),
                        stop=(k == KP - 1),
                    )
                init = (
                    0.0
                    if c == 0
                    else hs_tiles[c - 1][:, m, widths[c - 1] - 1 : widths[c - 1]]
                )
                nc.vector.tensor_tensor_scan(
                    out=hs_sb[:, m, :w],
                    data0=_bcast_free(acol[m], w),
                    data1=ps2[:, :w],
                    initial=init,
                    op0=mybir.AluOpType.mult,
                    op1=mybir.AluOpType.add,
                )
            hs_tiles[c] = hs_sb
        mm3_block(nch - 2)
        mm3_block(nch - 1)

    nc.finalize()
    _NC_CACHE[key] = nc
    return nc


def _pack128(w, kt):  # [kt*128, F] -> [128, kt*F]
    return np.transpose(w.reshape(kt, 128, -1), (1, 0, 2)).reshape(128, -1)


def _prep_shared(a, B, C, Wi, bi, Wo, bo):
    w2 = (C.astype(np.float64) @ Wo.astype(np.float64).T).astype(np.float32)
    # B m-major: [m0: k0 k1 | m1: k0 k1] so the kernel can split its load
    # into per-m configs
    bpk = _pack128(B, KP)  # [128, KP*H] k-major
    bpk = (
        bpk.reshape(128, KP, H // 128, 128)
        .transpose(0, 2, 1, 3)
        .reshape(128, KP * H)
    )
    return {
        "mpack": np.ascontiguousarray(
            np.concatenate(
                [bi.reshape(KP, 128).T, a.reshape(KH, 128).T, bo.reshape(MO, 128).T],
                axis=1,
            )
        ).astype(np.float32),
        "wb16": np.ascontiguousarray(
            np.concatenate([bpk, _pack128(w2, KH)], axis=1).astype(np.float16)
        ),
    }


def _pack_x(xb, widths):
    """[T, D] fp32 -> list of chunk-major [128, KD*w] fp16 blocks."""
    xT = xb.T.astype(np.float16)  # [D, T]
    blocks = []
    t0 = 0
    for w in widths:
        blocks.append(_pack128(xT[:, t0 : t0 + w], KD))
        t0 += w
    return blocks


def _unpack_out(r, widths):
    """chunk-major [128, MO*T] fp16 -> [T, O] fp32."""
    outT = np.empty((O, T), dtype=np.float16)
    t0 = 0
    for w in widths:
        blk = r[:, MO * t0 : MO * (t0 + w)].reshape(128, MO, w)
        outT[:, t0 : t0 + w] = blk.transpose(1, 0, 2).reshape(O, w)
        t0 += w
    return outT.T.astype(np.float32)


def kernel(x_seq, a, B, C, Wi, bi, Wo, bo, _collect=None):
    nc = build_nc()
    shared = _prep_shared(a, B, C, Wi, bi, Wo, bo)
    wi_blk = _pack128(np.ascontiguousarray(Wi.T).astype(np.float16), KD)
    in_maps = []
    for b in range(BSZ):
        m = dict(shared)
        blocks = _pack_x(x_seq[b], WIDTHS)
        # layout: [Wi k0 | ch0 | Wi k1-3 | ch1 | ch2 | ...]
        m["xw"] = np.ascontiguousarray(
            np.concatenate(
                [wi_blk[:, :P], blocks[0], wi_blk[:, P:]] + blocks[1:], axis=1
            )
        )
        in_maps.append(m)
    kwargs = {}
    if _collect is not None:
        kwargs = {k: v for k, v in _collect.items() if k != "res"}
    try:
        res = run_bass_kernel_spmd(nc, in_maps, core_ids=list(range(BSZ)), **kwargs)
    except Exception:
        # one retry for transient device errors
        res = run_bass_kernel_spmd(nc, in_maps, core_ids=list(range(BSZ)), **kwargs)
    if _collect is not None:
        _collect["res"] = res
    out = np.stack(
        [_unpack_out(res.results[b]["out"], WIDTHS) for b in range(BSZ)], axis=0
    )
    return out

